# revision 15
# baseline (speedup 1.0000x reference)
"""Trainium2 Bass kernel for the Cocoa contrastive loss.

loss = mean_i exp((1 - cos(x_i, y_i))/tau)
     + sum_{i in neg, j not in neg} exp(cos(x_i, x_j)/tau) / cnt
     + sum_{i in neg, j not in neg} exp(cos(y_i, y_j)/tau) / cnt

with neg = rows whose label has > 32 zeros, cnt = n_neg * n_nonneg.

Strategy (8 NeuronCores):
  Host: compute the neg mask (exact integer math), permute rows so neg rows
        come first, zero-pad the two groups to SPMD-friendly sizes.
  Phase 1 (data-parallel over 512 rows/core): row sum-of-squares (x on
        ScalarE Square-accum with the fp8 scale folded in, y on DVE
        mul+reduce), normalize straight to x*24/||x|| in fp8 e4m3 on DVE,
        raw per-row dots x_i . y_i for the pos term (product on GpSimd,
        accumulation on ScalarE), store the normalized fp8 rows in natural
        layout (4KB-contiguous descriptors).  No transposes on chip: the
        host's operand swizzle for phase 2 absorbs the transpose for free.
  Phase 2 (4x2 grid over neg x nonneg): fp8 DoubleRow GEMM
        sim*576 = A_neg @ B_nonneg^T with K=D on partitions, exp(sim/tau)
        on ScalarE with per-partition accumulation.  The moving operands
        are DMAed in two K-chunks and y's loads are deferred a few blocks
        so the PE gets a gap-free runway from ~8us on.
  Host: combine partial sums (subtract the exp(0)=1 contributions of the
        zero padding), compute pos term from the returned dots and norms
        in float64.
"""

import numpy as np
import ml_dtypes

import concourse.bass as bass
import concourse.bacc as bacc
import concourse.mybir as mybir
import concourse.tile as tile
from concourse.bass_utils import run_bass_kernel_spmd

TAU = 0.1
THRESHOLD = 32
B, D, L = 4096, 4096, 64
NCORES = 8
ROWS = B // NCORES  # 512 rows per core in phase 1
NGRP = ROWS // 128  # 4 row groups per core
KCH = D // 128      # 32 contraction chunks
A_SPLIT, B_SPLIT = 4, 2  # phase-2 core grid over (neg rows, nonneg rows)

F32 = mybir.dt.float32
BF16 = mybir.dt.bfloat16
FP8 = mybir.dt.float8e4
BF16_NP = ml_dtypes.bfloat16
FP8_NP = ml_dtypes.float8_e4m3fn
FP8_SCALE = 24.0  # centers N(0, 1/4096) values in e4m3's normal range
SCALE2 = FP8_SCALE * FP8_SCALE  # 576

# module-level caches so repeated kernel() calls don't rebuild/recompile
_CACHE: dict = {}

# filled in by the last kernel() call when tracing is enabled (test harness use)
LAST_RESULTS: list = []


def _build_phase1() -> bass.Bass:
    nc = bacc.Bacc(None)
    # [2, 128, 2, D] is byte-identical to [ROWS, D]: row = 256t + 2p + h.
    # Pairing two rows per partition doubles the load descriptors to 32KB.
    NT = 2
    x_in = nc.declare_dram_parameter("x", [NT, 128, 2, D], F32, isOutput=False)
    y_in = nc.declare_dram_parameter("y", [NT, 128, 2, D], F32, isOutput=False)
    xq_out = nc.declare_dram_parameter("xq", [NT, 128, 2, D], FP8, isOutput=True)
    yq_out = nc.declare_dram_parameter("yq", [NT, 128, 2, D], FP8, isOutput=True)

    with tile.TileContext(nc) as tc:
        with (
            tc.tile_pool(name="inp", bufs=4) as inp,
            tc.tile_pool(name="junk", bufs=3) as junkp,
            tc.tile_pool(name="out8", bufs=2) as outp,
            tc.tile_pool(name="small", bufs=1) as small,
        ):
            ssq = small.tile([128, NT, 2, 2], F32)   # ||row||^2/576
            invs = small.tile([128, NT, 2, 2], F32)  # 24/||row||

            # preload the Sqrt activation table during the DMA runway
            warm = small.tile([128, 1], F32)
            nc.vector.memset(warm, 1.0)
            nc.scalar.activation(warm, warm,
                                 mybir.ActivationFunctionType.Sqrt)

            # issue every input load up front; the DMA queues chew through
            # them in order while compute streams behind.
            tiles = {}
            for t in range(NT):
                for t_idx, src in ((0, x_in), (1, y_in)):
                    tg = inp.tile([128, 2, D], F32, tag="ld")
                    nc.sync.dma_start(out=tg, in_=src[t])
                    tiles[t_idx, t] = tg

            for t in range(NT):
                for t_idx, (src, dst) in enumerate(((x_in, xq_out),
                                                    (y_in, yq_out))):
                    tg = tiles[t_idx, t]
                    q = outp.tile([128, 2, D], FP8, tag="q")
                    for h in range(2):
                        # sumsq on Scalar: accum = sum((v/24)^2) = ||v||^2/576
                        jx = junkp.tile([128, D], BF16, tag="junk")
                        nc.scalar.activation(
                            jx, tg[:, h, :],
                            mybir.ActivationFunctionType.Square,
                            scale=1.0 / FP8_SCALE,
                            accum_out=ssq[:, t, t_idx, h:h + 1])
                        # sqrt -> ||.||/24 ; recip -> 24/||.||
                        nc.scalar.activation(
                            invs[:, t, t_idx, h:h + 1],
                            ssq[:, t, t_idx, h:h + 1],
                            mybir.ActivationFunctionType.Sqrt)
                        nc.vector.reciprocal(invs[:, t, t_idx, h:h + 1],
                                             invs[:, t, t_idx, h:h + 1])
                        # normalize straight to fp8: x * 24/||x||
                        nc.vector.tensor_scalar_mul(
                            q[:, h, :], tg[:, h, :],
                            invs[:, t, t_idx, h:h + 1])
                        nc.sync.dma_start(out=dst[t, :, h, :], in_=q[:, h, :])
    nc.compile()
    return nc


def _build_phase2(m_loc: int, n_loc: int) -> bass.Bass:
    """Per-core fp8 DoubleRow GEMM: [m_loc neg rows] x [n_loc nonneg rows].

    Operand roles are swapped vs the natural orientation: the nonneg side is
    the 128-wide stationary operand and the neg side is the 512-wide moving
    operand, so the matmul stream fully hides LDWEIGHTS.
    Host-supplied layouts (fully contiguous per DMA):
      l{x,y}: [128, KCH, m_loc]        moving side (neg rows)
      r{x,y}: [n_ch, 128, KCH, 128]    stationary side (nonneg rows)
    x's operands are loaded in K-chunks first for a quick gap-free PE start;
    y's loads are emitted a few stationary blocks into x's GEMM so they
    don't delay x's stationary prefetch.
    """
    nc = bacc.Bacc(None)
    n_ch = n_loc // 128
    n_ms = -(-m_loc // 512)  # moving sub-tiles of <=512
    assert m_loc % 16 == 0 and n_loc % 128 == 0
    lx = nc.declare_dram_parameter("lx", [128, KCH, m_loc], FP8, isOutput=False)
    rx = nc.declare_dram_parameter("rx", [n_ch, 128, KCH, 128], FP8, isOutput=False)
    ly = nc.declare_dram_parameter("ly", [128, KCH, m_loc], FP8, isOutput=False)
    ry = nc.declare_dram_parameter("ry", [n_ch, 128, KCH, 128], FP8, isOutput=False)
    acc_out = nc.declare_dram_parameter("acc", [128, 2 * n_ch * n_ms], F32,
                                        isOutput=True)

    msizes = [min(512, m_loc - 512 * i) for i in range(n_ms)]
    NCK = 2           # K-chunks for the latency-critical moving loads
    CK = KCH // NCK   # 16 kch per chunk

    with tile.TileContext(nc) as tc:
        with (
            tc.tile_pool(name="mov", bufs=1) as movp,
            tc.tile_pool(name="sta", bufs=4) as stap,
            tc.tile_pool(name="ps", bufs=4, space="PSUM") as psp,
            tc.tile_pool(name="junk", bufs=4) as junkp,
            tc.tile_pool(name="accp", bufs=1) as accp,
        ):
            acc = accp.tile([128, 2 * n_ch * n_ms], F32)
            # x runway: first stationary tile and moving operand in
            # interleaved K-chunks so the PE starts as early as possible
            st = {}
            s0ck = []
            lt = {"x": [], "y": []}
            for j in range(NCK):
                s = stap.tile([128, CK, 128], FP8, tag=f"st0x{j}",
                              name=f"st_x0_{j}")
                nc.sync.dma_start(out=s, in_=rx[0][:, j * CK:(j + 1) * CK, :])
                s0ck.append(s)
                t = movp.tile([128, CK, m_loc], FP8, tag=f"lx{j}",
                              name=f"lt_x{j}")
                nc.sync.dma_start(out=t, in_=lx[:, j * CK:(j + 1) * CK, :])
                lt["x"].append(t)
            st["x", 0] = s0ck

            def load_y():
                for j in range(NCK):
                    t = movp.tile([128, CK, m_loc], FP8, tag=f"ly{j}",
                                  name=f"lt_y{j}")
                    nc.sync.dma_start(out=t, in_=ly[:, j * CK:(j + 1) * CK, :])
                    lt["y"].append(t)
                s0y = stap.tile([128, KCH, 128], FP8, tag="st0y", name="st_y0")
                nc.sync.dma_start(out=s0y, in_=ry[0])
                st["y", 0] = s0y

            col = 0
            for name, rsrc in (("x", rx), ("y", ry)):
                for nch in range(n_ch):
                    if name == "x" and nch == min(3, n_ch - 1):
                        load_y()  # y loads queue behind x's first prefetches
                    s_t = None
                    s_ck = None
                    if (name, nch) in st:
                        s = st[name, nch]
                        if isinstance(s, list):
                            s_ck = s
                        else:
                            s_t = s
                    else:
                        s_t = stap.tile([128, KCH, 128], FP8, tag="st")
                        nc.sync.dma_start(out=s_t, in_=rsrc[nch])
                    for ms in range(n_ms):
                        msz = msizes[ms]
                        ps = psp.tile([128, 512], F32, tag="ps")
                        for kp in range(KCH // 2):
                            j, r = divmod(2 * kp, CK)
                            lhs = (s_ck[j][:, r:r + 2, :] if s_ck is not None
                                   else s_t[:, 2 * kp:2 * kp + 2, :])
                            nc.tensor.matmul(
                                ps[:, :msz],
                                lhsT=lhs,
                                rhs=lt[name][j][:, r:r + 2,
                                                512 * ms:512 * ms + msz],
                                start=(kp == 0), stop=(kp == KCH // 2 - 1),
                                perf_mode=mybir.MatmulPerfMode.DoubleRow)
                        jk = junkp.tile([128, 512], BF16, tag="junk")
                        nc.scalar.activation(
                            jk[:, :msz], ps[:, :msz],
                            mybir.ActivationFunctionType.Exp,
                            scale=1.0 / (TAU * SCALE2),
                            accum_out=acc[:, col:col + 1])
                        col += 1
            nc.sync.dma_start(out=acc_out[:], in_=acc)
    nc.compile()
    return nc


def _run_spmd(key, builder, in_maps):
    import os
    if key not in _CACHE:
        _CACHE[key] = builder()
    nc = _CACHE[key]
    trace = bool(os.environ.get("COCOA_TRACE"))
    res = run_bass_kernel_spmd(nc, in_maps, list(range(NCORES)), trace=trace)
    LAST_RESULTS.append((key, res))
    return res.results


def kernel(x_pred_batch: np.ndarray, y_pred_batch: np.ndarray,
           label_batch: np.ndarray) -> np.ndarray:
    x = np.ascontiguousarray(x_pred_batch, dtype=np.float32)
    y = np.ascontiguousarray(y_pred_batch, dtype=np.float32)
    lab = np.asarray(label_batch)

    # exact mask / permutation bookkeeping on host
    zero_counts = (lab == 0).sum(axis=1)
    neg_mask = zero_counts > THRESHOLD
    idx = np.concatenate([np.flatnonzero(neg_mask), np.flatnonzero(~neg_mask)])
    n1 = int(neg_mask.sum())
    n2 = B - n1
    cnt = n1 * n2

    xp = x[idx]
    yp = y[idx]

    # ---- phase 1 ----
    shp = (2, 128, 2, D)  # byte-identical view of [ROWS, D]
    in_maps = [
        {"x": xp[c * ROWS:(c + 1) * ROWS].reshape(shp),
         "y": yp[c * ROWS:(c + 1) * ROWS].reshape(shp)}
        for c in range(NCORES)
    ]
    res1 = _run_spmd("phase1", _build_phase1, in_maps)

    # pos term from the normalized fp8 rows (xq = x*24/||x|| in e4m3):
    # cos = sum(xq*yq)/576 per row, exp/mean in float64
    xq = np.concatenate([r["xq"].reshape(ROWS, D) for r in res1], axis=0)
    yq = np.concatenate([r["yq"].reshape(ROWS, D) for r in res1], axis=0)
    dots = np.einsum("ij,ij->i", xq.astype(np.float32),
                     yq.astype(np.float32), optimize=True).astype(np.float64)
    cos_pos = dots / SCALE2
    pos_error = float(np.mean(np.exp((1.0 - cos_pos) / TAU)))

    neg_total = 0.0
    if cnt > 0:
        # host transpose of the normalized fp8 rows -> [KCH, 128, B]
        xt = np.ascontiguousarray(xq.T).reshape(KCH, 128, B)
        yt = np.ascontiguousarray(yq.T).reshape(KCH, 128, B)

        m_loc = 16 * max(1, -(-n1 // (A_SPLIT * 16)))
        n_loc = 128 * max(1, -(-n2 // (B_SPLIT * 128)))
        n1p, n2p = A_SPLIT * m_loc, B_SPLIT * n_loc
        n_ch = n_loc // 128
        n_ms = -(-m_loc // 512)

        padded = {}
        for nm, t in (("x", xt), ("y", yt)):
            lhs = np.zeros((KCH, 128, n1p), FP8_NP)
            lhs[:, :, :n1] = t[:, :, :n1]
            rhs = np.zeros((KCH, 128, n2p), FP8_NP)
            rhs[:, :, :n2] = t[:, :, n1:]
            # swizzle to fully-contiguous per-DMA layouts (see _build_phase2)
            padded["l" + nm] = np.ascontiguousarray(lhs.transpose(1, 0, 2))
            padded["r" + nm] = np.ascontiguousarray(
                rhs.reshape(KCH, 128, B_SPLIT * n_ch, 128).transpose(2, 1, 0, 3))

        in_maps2 = []
        for c in range(NCORES):
            a, bgrid = divmod(c, B_SPLIT)
            cmap = {}
            for nm in ("x", "y"):
                cmap["l" + nm] = np.ascontiguousarray(
                    padded["l" + nm][:, :, a * m_loc:(a + 1) * m_loc])
                cmap["r" + nm] = padded["r" + nm][bgrid * n_ch:(bgrid + 1) * n_ch]
            in_maps2.append(cmap)

        res2 = _run_spmd(("phase2v4", m_loc, n_loc),
                         lambda: _build_phase2(m_loc, n_loc), in_maps2)

        n_half = n_ch * n_ms
        sx = sy = 0.0
        for r in res2:
            acc = r["acc"].astype(np.float64)
            sx += acc[:, :n_half].sum()
            sy += acc[:, n_half:].sum()
        pad = float(n1p) * n2p - float(n1) * n2
        neg_total = ((sx - pad) + (sy - pad)) / cnt

    return np.float32(pos_error + neg_total)


# revision 17
# speedup vs baseline: 1.0478x; 1.0478x over previous
"""Trainium2 Bass kernel for the Cocoa contrastive loss.

loss = mean_i exp((1 - cos(x_i, y_i))/tau)
     + sum_{i in neg, j not in neg} exp(cos(x_i, x_j)/tau) / cnt
     + sum_{i in neg, j not in neg} exp(cos(y_i, y_j)/tau) / cnt

with neg = rows whose label has > 32 zeros, cnt = n_neg * n_nonneg.

Strategy (8 NeuronCores):
  Host: compute the neg mask (exact integer math), permute rows so neg rows
        come first, zero-pad the two groups to SPMD-friendly sizes.
  Phase 1 (data-parallel over 512 rows/core): row sum-of-squares (x on
        ScalarE Square-accum with the fp8 scale folded in, y on DVE
        mul+reduce), normalize straight to x*24/||x|| in fp8 e4m3 on DVE,
        raw per-row dots x_i . y_i for the pos term (product on GpSimd,
        accumulation on ScalarE), store the normalized fp8 rows in natural
        layout (4KB-contiguous descriptors).  No transposes on chip: the
        host's operand swizzle for phase 2 absorbs the transpose for free.
  Phase 2 (4x2 grid over neg x nonneg): fp8 DoubleRow GEMM
        sim*576 = A_neg @ B_nonneg^T with K=D on partitions, exp(sim/tau)
        on ScalarE with per-partition accumulation.  The moving operands
        are DMAed in two K-chunks and y's loads are deferred a few blocks
        so the PE gets a gap-free runway from ~8us on.
  Host: combine partial sums (subtract the exp(0)=1 contributions of the
        zero padding), compute pos term from the returned dots and norms
        in float64.
"""

import numpy as np
import ml_dtypes

import concourse.bass as bass
import concourse.bacc as bacc
import concourse.mybir as mybir
import concourse.tile as tile
from concourse.bass_utils import run_bass_kernel_spmd

TAU = 0.1
THRESHOLD = 32
B, D, L = 4096, 4096, 64
NCORES = 8
ROWS = B // NCORES  # 512 rows per core in phase 1
NGRP = ROWS // 128  # 4 row groups per core
KCH = D // 128      # 32 contraction chunks
A_SPLIT, B_SPLIT = 4, 2  # phase-2 core grid over (neg rows, nonneg rows)

F32 = mybir.dt.float32
BF16 = mybir.dt.bfloat16
FP8 = mybir.dt.float8e4
BF16_NP = ml_dtypes.bfloat16
FP8_NP = ml_dtypes.float8_e4m3fn
FP8_SCALE = 24.0  # centers N(0, 1/4096) values in e4m3's normal range
SCALE2 = FP8_SCALE * FP8_SCALE  # 576

# module-level caches so repeated kernel() calls don't rebuild/recompile
_CACHE: dict = {}

# filled in by the last kernel() call when tracing is enabled (test harness use)
LAST_RESULTS: list = []


def _build_phase1() -> bass.Bass:
    nc = bacc.Bacc(None)
    x_in = nc.declare_dram_parameter("x", [ROWS, D], F32, isOutput=False)
    y_in = nc.declare_dram_parameter("y", [ROWS, D], F32, isOutput=False)
    xq_out = nc.declare_dram_parameter("xq", [ROWS, D], FP8, isOutput=True)
    yq_out = nc.declare_dram_parameter("yq", [ROWS, D], FP8, isOutput=True)

    with tile.TileContext(nc) as tc:
        with (
            tc.tile_pool(name="inp", bufs=8) as inp,
            tc.tile_pool(name="junk", bufs=3) as junkp,
            tc.tile_pool(name="out8", bufs=4) as outp,
            tc.tile_pool(name="small", bufs=1) as small,
        ):
            ssq = small.tile([128, NGRP, 2], F32)   # ||row||^2/576
            invs = small.tile([128, NGRP, 2], F32)  # 24/||row||

            # preload the Sqrt activation table during the DMA runway
            warm = small.tile([128, 1], F32)
            nc.vector.memset(warm, 1.0)
            nc.scalar.activation(warm, warm,
                                 mybir.ActivationFunctionType.Sqrt)

            # issue every input load up front; the DMA queues chew through
            # them in order while compute streams behind.
            tiles = {}
            for g in range(NGRP):
                for t_idx, src in ((0, x_in), (1, y_in)):
                    tg = inp.tile([128, D], F32, tag="ld")
                    nc.sync.dma_start(out=tg, in_=src[g * 128:(g + 1) * 128, :])
                    tiles[t_idx, g] = tg

            for g in range(NGRP):
                xg, yg = tiles[0, g], tiles[1, g]
                # sumsq on Scalar: accum = sum((v/24)^2) = ||v||^2/576
                for t_idx, tg in ((0, xg), (1, yg)):
                    jx = junkp.tile([128, D], BF16, tag="junk")
                    nc.scalar.activation(jx, tg,
                                         mybir.ActivationFunctionType.Square,
                                         scale=1.0 / FP8_SCALE,
                                         accum_out=ssq[:, g, t_idx:t_idx + 1])
                    # sqrt -> ||.||/24 ; recip -> 24/||.||
                    nc.scalar.activation(invs[:, g, t_idx:t_idx + 1],
                                         ssq[:, g, t_idx:t_idx + 1],
                                         mybir.ActivationFunctionType.Sqrt)
                nc.vector.reciprocal(invs[:, g, :], invs[:, g, :])

                # normalize straight to fp8: x * 24/||x||
                for t_idx, (tg, dst) in enumerate(((xg, xq_out), (yg, yq_out))):
                    q = outp.tile([128, D], FP8, tag="q")
                    nc.vector.tensor_scalar_mul(q, tg,
                                                invs[:, g, t_idx:t_idx + 1])
                    nc.sync.dma_start(out=dst[g * 128:(g + 1) * 128, :], in_=q)
    nc.compile()
    return nc


def _build_phase2(m_loc: int, n_loc: int) -> bass.Bass:
    """Per-core fp8 DoubleRow GEMM: [m_loc neg rows] x [n_loc nonneg rows].

    Operand roles are swapped vs the natural orientation: the nonneg side is
    the 128-wide stationary operand and the neg side is the 512-wide moving
    operand, so the matmul stream fully hides LDWEIGHTS.
    Host-supplied layouts (fully contiguous per DMA):
      l{x,y}: [128, KCH, m_loc]        moving side (neg rows)
      r{x,y}: [n_ch, 128, KCH, 128]    stationary side (nonneg rows)
    x's operands are loaded in K-chunks first for a quick gap-free PE start;
    y's loads are emitted a few stationary blocks into x's GEMM so they
    don't delay x's stationary prefetch.
    """
    nc = bacc.Bacc(None)
    n_ch = n_loc // 128
    n_ms = -(-m_loc // 512)  # moving sub-tiles of <=512
    assert m_loc % 16 == 0 and n_loc % 128 == 0
    lx = nc.declare_dram_parameter("lx", [128, KCH, m_loc], FP8, isOutput=False)
    rx = nc.declare_dram_parameter("rx", [n_ch, 128, KCH, 128], FP8, isOutput=False)
    ly = nc.declare_dram_parameter("ly", [128, KCH, m_loc], FP8, isOutput=False)
    ry = nc.declare_dram_parameter("ry", [n_ch, 128, KCH, 128], FP8, isOutput=False)
    acc_out = nc.declare_dram_parameter("acc", [128, 2 * n_ch * n_ms], F32,
                                        isOutput=True)

    msizes = [min(512, m_loc - 512 * i) for i in range(n_ms)]
    NCK = 2           # K-chunks for the latency-critical moving loads
    CK = KCH // NCK   # 16 kch per chunk

    with tile.TileContext(nc) as tc:
        with (
            tc.tile_pool(name="mov", bufs=1) as movp,
            tc.tile_pool(name="sta", bufs=4) as stap,
            tc.tile_pool(name="ps", bufs=4, space="PSUM") as psp,
            tc.tile_pool(name="junk", bufs=4) as junkp,
            tc.tile_pool(name="accp", bufs=1) as accp,
        ):
            acc = accp.tile([128, 2 * n_ch * n_ms], F32)
            # x runway: first stationary tile and moving operand in
            # interleaved K-chunks so the PE starts as early as possible
            st = {}
            s0ck = []
            lt = {"x": [], "y": []}
            for j in range(NCK):
                s = stap.tile([128, CK, 128], FP8, tag=f"st0x{j}",
                              name=f"st_x0_{j}")
                nc.sync.dma_start(out=s, in_=rx[0][:, j * CK:(j + 1) * CK, :])
                s0ck.append(s)
                t = movp.tile([128, CK, m_loc], FP8, tag=f"lx{j}",
                              name=f"lt_x{j}")
                nc.sync.dma_start(out=t, in_=lx[:, j * CK:(j + 1) * CK, :])
                lt["x"].append(t)
            st["x", 0] = s0ck

            def load_y():
                for j in range(NCK):
                    t = movp.tile([128, CK, m_loc], FP8, tag=f"ly{j}",
                                  name=f"lt_y{j}")
                    nc.sync.dma_start(out=t, in_=ly[:, j * CK:(j + 1) * CK, :])
                    lt["y"].append(t)
                s0y = stap.tile([128, KCH, 128], FP8, tag="st0y", name="st_y0")
                nc.sync.dma_start(out=s0y, in_=ry[0])
                st["y", 0] = s0y

            col = 0
            for name, rsrc in (("x", rx), ("y", ry)):
                for nch in range(n_ch):
                    if name == "x" and nch == min(3, n_ch - 1):
                        load_y()  # y loads queue behind x's first prefetches
                    s_t = None
                    s_ck = None
                    if (name, nch) in st:
                        s = st[name, nch]
                        if isinstance(s, list):
                            s_ck = s
                        else:
                            s_t = s
                    else:
                        s_t = stap.tile([128, KCH, 128], FP8, tag="st")
                        nc.sync.dma_start(out=s_t, in_=rsrc[nch])
                    for ms in range(n_ms):
                        msz = msizes[ms]
                        ps = psp.tile([128, 512], F32, tag="ps")
                        for kp in range(KCH // 2):
                            j, r = divmod(2 * kp, CK)
                            lhs = (s_ck[j][:, r:r + 2, :] if s_ck is not None
                                   else s_t[:, 2 * kp:2 * kp + 2, :])
                            nc.tensor.matmul(
                                ps[:, :msz],
                                lhsT=lhs,
                                rhs=lt[name][j][:, r:r + 2,
                                                512 * ms:512 * ms + msz],
                                start=(kp == 0), stop=(kp == KCH // 2 - 1),
                                perf_mode=mybir.MatmulPerfMode.DoubleRow)
                        jk = junkp.tile([128, 512], BF16, tag="junk")
                        nc.scalar.activation(
                            jk[:, :msz], ps[:, :msz],
                            mybir.ActivationFunctionType.Exp,
                            scale=1.0 / (TAU * SCALE2),
                            accum_out=acc[:, col:col + 1])
                        col += 1
            nc.sync.dma_start(out=acc_out[:], in_=acc)
    nc.compile()
    return nc


def _run_spmd(key, builder, in_maps):
    import os
    if key not in _CACHE:
        _CACHE[key] = builder()
    nc = _CACHE[key]
    trace = bool(os.environ.get("COCOA_TRACE"))
    res = run_bass_kernel_spmd(nc, in_maps, list(range(NCORES)), trace=trace)
    LAST_RESULTS.append((key, res))
    return res.results


def kernel(x_pred_batch: np.ndarray, y_pred_batch: np.ndarray,
           label_batch: np.ndarray) -> np.ndarray:
    x = np.ascontiguousarray(x_pred_batch, dtype=np.float32)
    y = np.ascontiguousarray(y_pred_batch, dtype=np.float32)
    lab = np.asarray(label_batch)

    # exact mask / permutation bookkeeping on host
    zero_counts = (lab == 0).sum(axis=1)
    neg_mask = zero_counts > THRESHOLD
    idx = np.concatenate([np.flatnonzero(neg_mask), np.flatnonzero(~neg_mask)])
    n1 = int(neg_mask.sum())
    n2 = B - n1
    cnt = n1 * n2

    xp = x[idx]
    yp = y[idx]

    # ---- phase 1 ----
    in_maps = [
        {"x": xp[c * ROWS:(c + 1) * ROWS], "y": yp[c * ROWS:(c + 1) * ROWS]}
        for c in range(NCORES)
    ]
    res1 = _run_spmd("phase1", _build_phase1, in_maps)

    # pos term from the normalized fp8 rows (xq = x*24/||x|| in e4m3):
    # cos = sum(xq*yq)/576 per row, exp/mean in float64
    xq = np.concatenate([r["xq"] for r in res1], axis=0)
    yq = np.concatenate([r["yq"] for r in res1], axis=0)
    dots = np.einsum("ij,ij->i", xq.astype(np.float32),
                     yq.astype(np.float32), optimize=True).astype(np.float64)
    cos_pos = dots / SCALE2
    pos_error = float(np.mean(np.exp((1.0 - cos_pos) / TAU)))

    neg_total = 0.0
    if cnt > 0:
        # host transpose of the normalized fp8 rows -> [KCH, 128, B]
        xt = np.ascontiguousarray(xq.T).reshape(KCH, 128, B)
        yt = np.ascontiguousarray(yq.T).reshape(KCH, 128, B)

        m_loc = 16 * max(1, -(-n1 // (A_SPLIT * 16)))
        n_loc = 128 * max(1, -(-n2 // (B_SPLIT * 128)))
        n1p, n2p = A_SPLIT * m_loc, B_SPLIT * n_loc
        n_ch = n_loc // 128
        n_ms = -(-m_loc // 512)

        padded = {}
        for nm, t in (("x", xt), ("y", yt)):
            lhs = np.zeros((KCH, 128, n1p), FP8_NP)
            lhs[:, :, :n1] = t[:, :, :n1]
            rhs = np.zeros((KCH, 128, n2p), FP8_NP)
            rhs[:, :, :n2] = t[:, :, n1:]
            # swizzle to fully-contiguous per-DMA layouts (see _build_phase2)
            padded["l" + nm] = np.ascontiguousarray(lhs.transpose(1, 0, 2))
            padded["r" + nm] = np.ascontiguousarray(
                rhs.reshape(KCH, 128, B_SPLIT * n_ch, 128).transpose(2, 1, 0, 3))

        in_maps2 = []
        for c in range(NCORES):
            a, bgrid = divmod(c, B_SPLIT)
            cmap = {}
            for nm in ("x", "y"):
                cmap["l" + nm] = np.ascontiguousarray(
                    padded["l" + nm][:, :, a * m_loc:(a + 1) * m_loc])
                cmap["r" + nm] = padded["r" + nm][bgrid * n_ch:(bgrid + 1) * n_ch]
            in_maps2.append(cmap)

        res2 = _run_spmd(("phase2v4", m_loc, n_loc),
                         lambda: _build_phase2(m_loc, n_loc), in_maps2)

        n_half = n_ch * n_ms
        sx = sy = 0.0
        for r in res2:
            acc = r["acc"].astype(np.float64)
            sx += acc[:, :n_half].sum()
            sy += acc[:, n_half:].sum()
        pad = float(n1p) * n2p - float(n1) * n2
        neg_total = ((sx - pad) + (sy - pad)) / cnt

    return np.float32(pos_error + neg_total)
